# revision 5
# baseline (speedup 1.0000x reference)
"""Trainium2 Bass kernel for ViT window attention with relative position bias.

Full inputs in, full outputs out. Data-parallel over batch: 64 batches split
8 per NeuronCore, weights replicated, no collectives.

Self-contained: hardcodes shapes and the (deterministic) relative-position
index structure; builds + compiles the Bass graph once per process.
"""

import os
import sys

for _p in ("/opt/trn_rl_repo", "/root/.axon_site/_ro/trn_rl_repo"):
    if os.path.isdir(_p) and _p not in sys.path:
        sys.path.insert(0, _p)

import numpy as np

import concourse.bass as bass
import concourse.mybir as mybir
import concourse.tile as tile
from concourse import bacc
from concourse.bass import AP
from concourse.masks import make_identity

F32 = mybir.dt.float32
BF16 = mybir.dt.bfloat16
AF = mybir.ActivationFunctionType

# problem constants
WIN = 14
NSEQ = WIN * WIN + 1          # 197
H = 16                        # heads
HD = 64                       # head dim
C = 1024
NREL = (2 * WIN - 1) * (2 * WIN - 1) + 3   # 732
B_FULL = 64
BC = 8                        # batches per core
T = BC * NSEQ                 # 1576 tokens per core
SCALE = HD ** -0.5            # 0.125
TCH = 394                     # qkv t-chunk (4 * 394 = 1576, fits one psum bank)
NT_TILE = 13                  # ceil(1576 / 128)
TPITCH = NT_TILE * 128        # 1664, xT free extent


def build_nc(sim: bool = False, stage: int = 99):
    nc = _build_graph(sim, stage)
    nc.compile()
    return nc


def _build_graph(sim: bool = False, stage: int = 99):
    if sim:
        nc = bacc.Bacc(None, target_bir_lowering=False, debug=True)
    else:
        nc = bacc.Bacc(None)

    def dump(nc, out_ext, ap, row0):
        nc.gpsimd.dma_start(out_ext[row0: row0 + ap.shape[0], 0: ap.free_size()], ap)

    x_ext = nc.declare_dram_parameter("x", [T, C], F32, isOutput=False)
    qkvw_ext = nc.declare_dram_parameter("qkv_w", [3 * C, C], F32, isOutput=False)
    qb_ext = nc.declare_dram_parameter("q_bias", [1, C], F32, isOutput=False)
    vb_ext = nc.declare_dram_parameter("v_bias", [1, C], F32, isOutput=False)
    rpb_ext = nc.declare_dram_parameter("rpb_table", [NREL, H], F32, isOutput=False)
    pw_ext = nc.declare_dram_parameter("proj_w", [C, C], F32, isOutput=False)
    pb_ext = nc.declare_dram_parameter("proj_b", [1, C], F32, isOutput=False)
    out_ext = nc.declare_dram_parameter("out", [T, C], F32, isOutput=True)

    # permutation constants: window stacks are loaded in j-major ascending
    # order from the shifted-table copies; these map load-rows to key order.
    sig_a = np.zeros(128, np.int64)
    for ki in range(9):
        for kj in range(14):
            # key row 1+14ki+kj needs off=364-27ki-kj = 135+27(8-ki)+(13-kj)
            sig_a[1 + 14 * ki + kj] = 1 + 9 * (13 - kj) + (8 - ki)
    sig_a[0] = 0
    sig_a[127] = 127
    sig_b = np.zeros(69, np.int64)
    for i in range(13):
        # key row i (kk=127+i) needs off=120-i = tabHb[4, 13+(12-i)... load
        # piece1 row j holds off 108+j -> j = 12 - i
        sig_b[i] = 12 - i
    for m in range(4):
        for kj in range(14):
            # key row 13+14m+kj needs off = 94-27m-kj = 27(3-m)+(13-kj)
            sig_b[13 + 14 * m + kj] = 13 + 4 * (13 - kj) + (3 - m)
    pa_np = np.zeros((128, 128), np.float32)
    pa_np[sig_a, np.arange(128)] = 1.0
    pb_np = np.zeros((69, 69), np.float32)
    pb_np[sig_b, np.arange(69)] = 1.0
    import ml_dtypes
    prev_a_d = nc.inline_tensor(pa_np.astype(ml_dtypes.bfloat16), name="prev_a")
    prev_b_d = nc.inline_tensor(pb_np.astype(ml_dtypes.bfloat16), name="prev_b")

    # DRAM staging tensors for the shifted table copies (exp'ed values)
    tabH_d = nc.dram_tensor("tabH_d", [H * 9, 379], BF16)
    tabHb_d = nc.dram_tensor("tabHb_d", [H * 5, 378], BF16)

    with tile.TileContext(nc) as tc:
        with tc.tile_pool(name="persist", bufs=1) as pp:
            ident_bf = pp.tile([128, 128], BF16, name="ident_bf", tag="ident_bf")
            make_identity(nc, ident_bf)
            ident_f = pp.tile([128, 128], F32, name="ident_f", tag="ident_f")
            make_identity(nc, ident_f)
            ones_row = pp.tile([1, 128], BF16, name="ones_row", tag="ones_row")
            nc.gpsimd.memset(ones_row[:], 1.0)
            onesK = pp.tile([128, HD], BF16, name="onesK", tag="onesK")
            nc.gpsimd.memset(onesK[:], 1.0)

            tabT = pp.tile([H, 736], BF16, name="tabT", tag="tabT")
            eb729 = pp.tile([128, H], BF16, name="eb729", tag="eb729")
            qb_col = pp.tile([128, 8], F32, name="qb_col", tag="qb_col")
            qb_row = pp.tile([1, C], F32, name="qb_row", tag="qb_row")
            vb_row = pp.tile([1, C], BF16, name="vb_row", tag="vb_row")
            pb_row = pp.tile([1, C], BF16, name="pb_row", tag="pb_row")
            vb_bcast = pp.tile([128, C], F32, name="vb_bcast", tag="vb_bcast")
            prev_a = pp.tile([128, 128], BF16, name="prev_a_s", tag="prev_a_s")
            prev_b = pp.tile([69, 69], BF16, name="prev_b_s", tag="prev_b_s")
            erow = []

            # persistent outputs of phase 1
            QKT = [
                pp.tile([128, T], BF16, name=f"qkt{ot}", tag=f"qkt{ot}")
                for ot in range(16)
            ]
            V1 = {}
            for b in range(BC):
                V1[(b, 0)] = pp.tile([128, H * HD], BF16,
                                     name=f"v1_{b}_0", tag=f"v1_{b}_0")
                V1[(b, 1)] = pp.tile([69, H * HD], BF16,
                                     name=f"v1_{b}_1", tag=f"v1_{b}_1")
            EBa = [pp.tile([128, NSEQ], BF16, name=f"eba{h}", tag=f"eba{h}")
                   for h in range(H)]
            EBb = [pp.tile([69, NSEQ], BF16, name=f"ebb{h}", tag=f"ebb{h}")
                   for h in range(H)]

            with (
                tc.tile_pool(name="xw", bufs=1) as xw,
                tc.tile_pool(name="stage", bufs=6) as stg,
                tc.tile_pool(name="cast", bufs=6) as cst,
                tc.tile_pool(name="wpad", bufs=8) as wpool,
                tc.tile_pool(name="ps_t", bufs=4, space="PSUM") as ps_t,
                tc.tile_pool(name="psum_mm", bufs=4, space="PSUM") as psmm,
            ):
                xT = [
                    xw.tile([128, TPITCH], BF16, name=f"xt{ct}", tag=f"xt{ct}")
                    for ct in range(8)
                ]
                WT = [
                    xw.tile([128, 3 * C], BF16, name=f"wt{ct}", tag=f"wt{ct}")
                    for ct in range(8)
                ]

                # ---- streaming load machinery -----------------------------
                # DMA queues, cast engines and copy-back engines round-robin
                # so no single engine serializes the load pipeline.
                dmaq = [nc.sync, nc.gpsimd, nc.scalar]
                ldq = [0]
                castq = [0]
                cbq = [0]

                def cast_eng():
                    e = (nc.vector, nc.scalar)[castq[0] % 2]
                    castq[0] += 1
                    return e

                def cb_copy(dst, src):
                    i = cbq[0] % 3
                    cbq[0] += 1
                    if i == 0:
                        nc.vector.tensor_copy(dst, src)
                    elif i == 1:
                        nc.scalar.copy(dst, src)
                    else:
                        nc.vector.tensor_copy(dst, src)

                def load_block(dst_list, src_ext, ob, dst_col):
                    rsz = min(128, src_ext.shape[0] - ob * 128)
                    for half in range(2):
                        sb = stg.tile([128, 512], F32, name="xstage",
                                      tag="xstage")
                        if rsz < 128:
                            nc.gpsimd.memset(sb[:], 0.0)
                        eng = dmaq[ldq[0] % 3]
                        ldq[0] += 1
                        eng.dma_start(
                            sb[0:rsz, :],
                            src_ext[ob * 128: ob * 128 + rsz,
                                    half * 512:(half + 1) * 512],
                        )
                        sbf = cst.tile([128, 512], BF16, name="bstage",
                                       tag="bstage")
                        ce = cast_eng()
                        if ce is nc.scalar:
                            ce.copy(sbf[:], sb[:])
                        else:
                            ce.tensor_copy(sbf[:], sb[:])
                        for ci in range(4):
                            ct = half * 4 + ci
                            ptr = ps_t.tile([128, 128], BF16, name="ptr",
                                            tag="ptr")
                            nc.tensor.transpose(
                                ptr[:], sbf[:, ci * 128:(ci + 1) * 128],
                                ident_bf[:]
                            )
                            dst = dst_list[ct][:, dst_col + ob * 128:
                                               dst_col + (ob + 1) * 128]
                            cb_copy(dst, ptr[:])

                # issue the first loads before anything else so DMA queues
                # fill from t=0 (b1 chunk 0 needs x0..3 + W0)
                for tt in range(4):
                    load_block(xT, x_ext, tt, 0)
                load_block(WT, qkvw_ext, 0, 0)
                load_block(WT, qkvw_ext, 1, 0)

                # ---- table prep (exp applied to the table ONCE, here) -----
                if True:
                    for j in range(6):
                        rs = min(128, NREL - j * 128)
                        tbj = pp.tile([128, H], F32, name="tbstage",
                                      tag="tbstage")
                        nc.gpsimd.dma_start(tbj[0:rs, :],
                                            rpb_ext[j * 128: j * 128 + rs, :])
                        ptp = ps_t.tile([H, 128], F32, name="tp", tag="ptr")
                        nc.tensor.transpose(ptp[:, 0:rs], tbj[0:rs, :],
                                            ident_f[0:rs, 0:rs])
                        nc.scalar.activation(
                            tabT[:, j * 128: j * 128 + rs], ptp[:, 0:rs],
                            AF.Exp,
                        )
                    nc.gpsimd.dma_start(prev_a[:], prev_a_d[:])
                    nc.gpsimd.dma_start(prev_b[:], prev_b_d[:])
                    # shifted copies: tabH_d[h*9+m, s] = tab[h, 135+27m+s],
                    #                 tabHb_d[h*5+m2, s] = tab[h, 27*m2+s]
                    tapc = tabT[:]
                    with nc.allow_non_contiguous_dma("shifted table copies"):
                        nc.gpsimd.dma_start(
                            tabH_d[:],
                            AP(tapc.tensor, tapc.offset + 135,
                               [[736, H], [27, 9], [1, 379]]),
                        )
                        nc.gpsimd.dma_start(
                            tabHb_d[:],
                            AP(tapc.tensor, tapc.offset,
                               [[736, H], [27, 5], [1, 378]]),
                        )

                    for j in range(3):
                        pte = ps_t.tile([1, H], BF16, name="te", tag="ptr")
                        nc.tensor.transpose(
                            pte[:], tabT[0:H, 729 + j: 730 + j],
                            ident_bf[0:H, 0:H]
                        )
                        er = pp.tile([1, H], BF16, name=f"erow{j}",
                                     tag=f"erow{j}")
                        nc.vector.tensor_copy(er[:], pte[:])
                        erow.append(er)
                    p729 = psmm.tile([128, H], F32, name="p729", tag="pq")
                    nc.tensor.matmul(p729[:], ones_row[:], erow[0][:],
                                     start=True, stop=True)
                    nc.scalar.copy(eb729[:], p729[:])

                    nc.sync.dma_start(qb_row[:], qb_ext[:])
                    nc.gpsimd.dma_start(vb_row[:], vb_ext[:])
                    nc.gpsimd.dma_start(pb_row[:], pb_ext[:])
                    # qb_col[p, ot] = q_bias[ot*128 + p] via 8 PE transposes
                    for ot in range(8):
                        ptq = ps_t.tile([128, 1], F32, name="tq", tag="ptr")
                        nc.tensor.transpose(
                            ptq[:], qb_row[0:1, ot * 128:(ot + 1) * 128],
                            ident_f[0:1, 0:1],
                        )
                        nc.vector.tensor_copy(qb_col[:, ot: ot + 1], ptq[:])
                    for oc in range(2):
                        pvb = psmm.tile([128, 512], F32, name="pbc", tag="pq")
                        nc.tensor.matmul(
                            pvb[:], ones_row[:],
                            vb_row[:, oc * 512:(oc + 1) * 512],
                            start=True, stop=True,
                        )
                        nc.scalar.copy(vb_bcast[:, oc * 512:(oc + 1) * 512],
                                       pvb[:])

                # --- interleaved EB machinery ------------------------------
                win_sets = []   # (wtmp, np_, prv, ebt, h, is_a)
                win_jobs = [(h, a) for h in range(H) for a in (True, False)]
                wdq = [0]

                def emit_window_set(job):
                    (h, is_a) = job
                    weng = (nc.gpsimd, nc.sync)[wdq[0] % 2]
                    wdq[0] += 1
                    wtmp = wpool.tile([128, 365], BF16, name="wtmp", tag="wtmp")
                    with nc.allow_non_contiguous_dma("toeplitz windows"):
                        if is_a:
                            # rows 1..126 j-major from tabH_d
                            weng.dma_start(
                                wtmp[1:127, :],
                                AP(tabH_d, h * 9 * 379,
                                   [[1, WIN], [379, 9], [1, 365]]),
                            )
                            # row 0 (dummy) + row 127 (off=121=tabHb[4,13])
                            wap = wtmp[:]
                            weng.dma_start(
                                AP(wap.tensor, wap.offset,
                                   [[365 * 127, 2], [1, 365]]),
                                AP(tabHb_d, (h * 5 + 4) * 378,
                                   [[13, 2], [1, 365]]),
                            )
                            np_, prv = 128, prev_a
                        else:
                            weng.dma_start(
                                wtmp[0:13, :],
                                AP(tabHb_d, (h * 5 + 4) * 378,
                                   [[1, 13], [1, 365]]),
                            )
                            weng.dma_start(
                                wtmp[13:69, :],
                                AP(tabHb_d, h * 5 * 378,
                                   [[1, WIN], [378, 4], [1, 365]]),
                            )
                            np_, prv = 69, prev_b
                    ebt = EBa[h] if is_a else EBb[h]
                    win_sets.append((wtmp, np_, prv, ebt, h, is_a))

                wrq = [0]

                def emit_wrev_extract(i):
                    (wtmp, np_, prv, ebt, h, is_a) = win_sets[i]
                    wrev = psmm.tile([128, 365], F32, name="wrev", tag="pq")
                    nc.tensor.matmul(
                        wrev[0:np_, :], prv[0:np_, 0:np_], wtmp[0:np_, :],
                        start=True, stop=True,
                    )
                    wap = wrev[0:np_, :]
                    ebtap = ebt[:]
                    # table already exp'ed: extraction is a plain copy
                    xeng = (nc.vector, nc.scalar)[wrq[0] % 2]
                    wrq[0] += 1
                    dst_ap = AP(ebtap.tensor, ebtap.offset + 1,
                                [[NSEQ, np_], [WIN, WIN], [1, WIN]])
                    src_ap = AP(wap.tensor, wap.offset,
                                [[365, np_], [27, WIN], [1, WIN]])
                    if xeng is nc.scalar:
                        xeng.copy(dst_ap, src_ap)
                    else:
                        xeng.tensor_copy(dst_ap, src_ap)
                    nc.vector.tensor_copy(
                        ebt[0:np_, 0:1], eb729[0:np_, h: h + 1]
                    )
                    if not is_a:  # after the b-tile, patch the CLS key row
                        e1 = erow[1][:]
                        nc.vector.tensor_copy(
                            EBa[h][0:1, 1:NSEQ],
                            AP(e1.tensor, e1.offset + h,
                               [[H, 1], [0, NSEQ - 1]]),
                        )
                        nc.vector.tensor_copy(
                            EBa[h][0:1, 0:1], erow[2][0:1, h: h + 1]
                        )

                wctr = [0]
                wrctr = [0]

                def b1_chunk(ot, tci):
                    t0 = tci * TCH
                    pq = psmm.tile([128, 512], F32, name="pq", tag="pq")
                    for ct in range(8):
                        nc.tensor.matmul(
                            pq[:, 0:TCH],
                            WT[ct][:, ot * 128:(ot + 1) * 128],
                            xT[ct][:, t0: t0 + TCH],
                            start=(ct == 0), stop=(ct == 7),
                        )
                    if ot < 8:
                        nc.vector.tensor_scalar_add(
                            QKT[ot][:, t0: t0 + TCH], pq[:, 0:TCH],
                            qb_col[:, ot: ot + 1],
                        )
                    else:
                        nc.vector.tensor_copy(
                            QKT[ot][:, t0: t0 + TCH], pq[:, 0:TCH]
                        )
                    if wctr[0] < len(win_jobs):
                        emit_window_set(win_jobs[wctr[0]])
                        wctr[0] += 1
                    if wrctr[0] + 8 < wctr[0] and wrctr[0] < len(win_jobs):
                        emit_wrev_extract(wrctr[0])
                        wrctr[0] += 1

                def b1_otile(ot):
                    for tci in range(4):
                        b1_chunk(ot, tci)

                # x-tiles needed per 394-token chunk of B1(ot=0):
                # chunk 0 -> x0..3, 1 -> x4..6(+3), 2 -> x7..9, 3 -> x10..12
                b1_chunk(0, 0)
                for ci, xts in enumerate(((4, 5, 6), (7, 8, 9), (10, 11, 12))):
                    for tt in xts:
                        load_block(xT, x_ext, tt, 0)
                    b1_chunk(0, ci + 1)
                b1_chunk(1, 0)
                b1_chunk(1, 1)
                for ob in range(2, 24):
                    load_block(WT, qkvw_ext, ob, 0)
                    if ob < 16:
                        b1_otile(ob)
                b1_chunk(1, 2)
                b1_chunk(1, 3)
                while wctr[0] < len(win_jobs):
                    emit_window_set(win_jobs[wctr[0]])
                    wctr[0] += 1

                if stage <= 2:
                    dump(nc, out_ext, xT[0][:, 0:1024], 0)
                    dump(nc, out_ext, WT[0][:, 0:1024], 128)
                    return nc

                # V projection -> V1[(b,kt)] bf16 (+bias)
                for b in range(BC):
                    for kt, (ko, ksz) in enumerate(((0, 128), (128, 69))):
                        t0 = b * NSEQ + ko
                        v1t = V1[(b, kt)]
                        for oc in range(2):
                            pv = psmm.tile([128, 512], F32, name="pv", tag="pq")
                            for ct in range(8):
                                nc.tensor.matmul(
                                    pv[0:ksz, :],
                                    xT[ct][:, t0: t0 + ksz],
                                    WT[ct][:, 2 * C + oc * 512:
                                            2 * C + (oc + 1) * 512],
                                    start=(ct == 0), stop=(ct == 7),
                                )
                            nc.vector.tensor_add(
                                v1t[0:ksz, oc * 512:(oc + 1) * 512],
                                pv[0:ksz, :],
                                vb_bcast[0:ksz, oc * 512:(oc + 1) * 512],
                            )
                        if wrctr[0] < len(win_sets):
                            emit_wrev_extract(wrctr[0])
                            wrctr[0] += 1
                        if wrctr[0] < len(win_sets):
                            emit_wrev_extract(wrctr[0])
                            wrctr[0] += 1
                while wrctr[0] < len(win_sets):
                    emit_wrev_extract(wrctr[0])
                    wrctr[0] += 1

            if stage <= 1 or stage == 4:
                for hh in range(4):
                    dump(nc, out_ext, EBa[hh][:, :], hh * 128)
                return nc

            # ----- phase 2: attention + output projection -------------------
            from contextlib import ExitStack
            with (
                tc.tile_pool(name="attbig", bufs=1) as ab,
                tc.tile_pool(name="stage2", bufs=6) as stage2,
                tc.tile_pool(name="stage2b", bufs=6) as stage2b,
                tc.tile_pool(name="attw", bufs=8) as attw,
                tc.tile_pool(name="attsmall", bufs=6) as attsmall,
                tc.tile_pool(name="ostage", bufs=3) as op_,
            ):
                att_ctx = ExitStack()
                ps_s = att_ctx.enter_context(
                    tc.tile_pool(name="ps_s", bufs=4, space="PSUM"))
                ps_o = att_ctx.enter_context(
                    tc.tile_pool(name="ps_o", bufs=2, space="PSUM"))
                ps_r = att_ctx.enter_context(
                    tc.tile_pool(name="ps_r", bufs=2, space="PSUM"))
                ps_j = ps_r
                attnT = [
                    ab.tile([128, T], BF16, name=f"at{ct}", tag=f"at{ct}")
                    for ct in range(8)
                ]
                PWT = [
                    ab.tile([128, C], BF16, name=f"pwt{ct}", tag=f"pwt{ct}")
                    for ct in range(8)
                ]
                pb_bcast = ab.tile([128, C], F32, name="pb_bcast", tag="pb_bcast")
                for oc in range(2):
                    ppb = ps_r.tile([128, 512], F32, name="pbc2", tag="rbp")
                    nc.tensor.matmul(
                        ppb[:], ones_row[:], pb_row[:, oc * 512:(oc + 1) * 512],
                        start=True, stop=True,
                    )
                    nc.scalar.copy(pb_bcast[:, oc * 512:(oc + 1) * 512], ppb[:])

                pw_jobs = []
                pwdq = [0]
                for ob in range(8):
                    for half in range(2):
                        pwb = stage2.tile([128, 512], F32, name="pwstage",
                                          tag="pwstage")
                        peng = dmaq[pwdq[0] % 3]
                        pwdq[0] += 1
                        peng.dma_start(
                            pwb[:],
                            pw_ext[ob * 128:(ob + 1) * 128,
                                   half * 512:(half + 1) * 512])
                        pwbf = stage2b.tile([128, 512], BF16, name="pwbstage",
                                            tag="pwbstage")
                        nc.vector.tensor_copy(pwbf[:], pwb[:])
                        for ci in range(4):
                            pw_jobs.append((pwbf, ob, half * 4 + ci))
                # stage2 pool must keep pwbf tiles alive until their
                # transposes are emitted (spread through early attention)
                pwctr = [0]

                def emit_pw_transposes(n):
                    while n > 0 and pwctr[0] < len(pw_jobs):
                        (pwbf, ob, ct) = pw_jobs[pwctr[0]]
                        ci = ct % 4
                        pwctr[0] += 1
                        n -= 1
                        ptr = ps_r.tile([128, 128], BF16, name="ptr2", tag="rbp")
                        nc.tensor.transpose(
                            ptr[:], pwbf[:, ci * 128:(ci + 1) * 128], ident_bf[:]
                        )
                        dst = PWT[ct][:, ob * 128:(ob + 1) * 128]
                        if (ob + ct) % 2 == 0:
                            nc.scalar.copy(dst, ptr[:])
                        else:
                            nc.vector.tensor_copy(dst, ptr[:])

                emit_pw_transposes(24)

                def emit_b4(tt):
                    tsz = min(128, T - tt * 128)
                    ost = op_.tile([128, C], F32, name="ost", tag="ost")
                    for oc in range(2):
                        pp2 = ps_j.tile([128, 512], F32, name="ppj", tag="rbp")
                        for ct in range(8):
                            nc.tensor.matmul(
                                pp2[0:tsz, :],
                                attnT[ct][:, tt * 128: tt * 128 + tsz],
                                PWT[ct][:, oc * 512:(oc + 1) * 512],
                                start=(ct == 0), stop=(ct == 7),
                            )
                        nc.vector.tensor_add(
                            ost[0:tsz, oc * 512:(oc + 1) * 512],
                            pp2[0:tsz, :],
                            pb_bcast[0:tsz, oc * 512:(oc + 1) * 512],
                        )
                    nc.sync.dma_start(
                        out_ext[tt * 128: tt * 128 + tsz, :], ost[0:tsz, :]
                    )

                b4ctr = [0]
                # attention: head-paired column-tiled PV/normalize
                for bp in range(BC // 2):
                    t0p = 2 * bp * NSEQ
                    for hp in range(8):
                        po = ps_o.tile([128, 2 * NSEQ], F32, name="po",
                                       tag="po", padded_shape=[128, 512])
                        rbp = ps_r.tile([128, 2 * NSEQ], F32, name="rbp",
                                        tag="rbp", padded_shape=[128, 512])
                        for hh in range(2):
                            h = 2 * hp + hh
                            qpo = 64 * hh
                            cpos = 64 * hh
                            kot = 8 + hp
                            pts = []
                            for kt, (ko, ksz) in enumerate(((0, 128),
                                                            (128, 69))):
                                ps = ps_s.tile([128, 2 * NSEQ], F32,
                                               name="ps", tag="ps")
                                for bi in range(2):
                                    b = 2 * bp + bi
                                    t0 = b * NSEQ
                                    nc.tensor.matmul(
                                        ps[0:ksz, bi * NSEQ:(bi + 1) * NSEQ],
                                        QKT[kot][qpo: qpo + 64,
                                                 t0 + ko: t0 + ko + ksz],
                                        QKT[hp][qpo: qpo + 64, t0: t0 + NSEQ],
                                        start=True, stop=True,
                                    )
                                pt = attw.tile([128, 2 * NSEQ], BF16,
                                               name="pt", tag="pt")
                                nc.scalar.activation(
                                    pt[0:ksz, :], ps[0:ksz, :], AF.Exp,
                                    scale=SCALE,
                                )
                                eb = (EBa[h] if kt == 0 else EBb[h])
                                for (meng, bi2) in ((nc.vector, 0),
                                                    (nc.gpsimd, 1)):
                                    meng.tensor_mul(
                                        pt[0:ksz, bi2 * NSEQ:(bi2 + 1) * NSEQ],
                                        pt[0:ksz, bi2 * NSEQ:(bi2 + 1) * NSEQ],
                                        eb[0:ksz, :],
                                    )
                                pts.append((pt, ksz))
                                nc.tensor.matmul(
                                    rbp[cpos: cpos + 64, :],
                                    onesK[0:ksz, :], pt[0:ksz, :],
                                    start=(kt == 0), stop=(kt == 1),
                                    tile_position=(0, cpos),
                                )
                            for bi in range(2):
                                b = 2 * bp + bi
                                for kt, (ko, ksz) in enumerate(((0, 128),
                                                                (128, 69))):
                                    nc.tensor.matmul(
                                        po[cpos: cpos + 64,
                                           bi * NSEQ:(bi + 1) * NSEQ],
                                        V1[(b, kt)][0:ksz,
                                                    h * HD:(h + 1) * HD],
                                        pts[kt][0][0:ksz,
                                                   bi * NSEQ:(bi + 1) * NSEQ],
                                        start=(kt == 0), stop=(kt == 1),
                                        tile_position=(0, cpos),
                                    )
                        rbs = attsmall.tile([128, 2 * NSEQ], F32,
                                            name="rbs", tag="rbs")
                        nc.vector.reciprocal_approx_fast(rbs[:], rbp[:])
                        nc.vector.tensor_mul(
                            attnT[hp][:, t0p: t0p + 2 * NSEQ],
                            po[:],
                            rbs[:],
                        )
                        emit_pw_transposes(8)
                    while (b4ctr[0] + 1) * 128 <= (bp + 1) * 2 * NSEQ:
                        emit_b4(b4ctr[0])
                        b4ctr[0] += 1

                if stage <= 5:
                    dump(nc, out_ext, attnT[0][:, 0:1024], 0)
                    return nc

                # flush remaining output-projection tiles
                while b4ctr[0] < NT_TILE:
                    emit_b4(b4ctr[0])
                    b4ctr[0] += 1
                att_ctx.close()

    return nc


_NC = None
LAST_RESULT = None


def _get_nc():
    global _NC
    if _NC is None:
        _NC = build_nc()
    return _NC


def make_in_maps(x, qkv_w, q_bias, v_bias, rpb_table, proj_w, proj_b):
    x = np.ascontiguousarray(np.asarray(x, np.float32))
    qkv_w = np.ascontiguousarray(np.asarray(qkv_w, np.float32))
    q_bias = np.ascontiguousarray(np.asarray(q_bias, np.float32).reshape(1, C))
    v_bias = np.ascontiguousarray(np.asarray(v_bias, np.float32).reshape(1, C))
    rpb_table = np.ascontiguousarray(np.asarray(rpb_table, np.float32))
    proj_w = np.ascontiguousarray(np.asarray(proj_w, np.float32))
    proj_b = np.ascontiguousarray(np.asarray(proj_b, np.float32).reshape(1, C))
    in_maps = []
    for c in range(8):
        xs = np.ascontiguousarray(
            x[c * BC:(c + 1) * BC].reshape(T, C)
        )
        in_maps.append({
            "x": xs, "qkv_w": qkv_w, "q_bias": q_bias, "v_bias": v_bias,
            "rpb_table": rpb_table, "proj_w": proj_w, "proj_b": proj_b,
        })
    return in_maps


def _ensure_axon_hooks_importable():
    """bass_utils imports antenv.axon_hooks when BASS_TRACE is set; the image's
    antenv lacks that module. Provide a no-op stand-in so tracing degrades
    gracefully instead of crashing (unless a real one is already installed)."""
    import types
    try:
        import antenv.axon_hooks  # noqa: F401
    except Exception:
        mod = types.ModuleType("antenv.axon_hooks")
        mod._h = None
        mod.set_axon_ntff_profile_hook = lambda h: setattr(mod, "_h", h)
        mod.get_axon_ntff_profile_hook = lambda: mod._h
        sys.modules["antenv.axon_hooks"] = mod
        try:
            import antenv
            antenv.axon_hooks = mod
        except Exception:
            pass


def kernel(x, qkv_w, q_bias, v_bias, rpb_table, proj_w, proj_b,
           rel_pos_index=None, **_unused):
    global LAST_RESULT
    _ensure_axon_hooks_importable()
    from concourse.bass_utils import run_bass_kernel_spmd

    nc = _get_nc()
    in_maps = make_in_maps(x, qkv_w, q_bias, v_bias, rpb_table, proj_w, proj_b)
    res = run_bass_kernel_spmd(nc, in_maps, core_ids=list(range(8)))
    LAST_RESULT = res
    out = np.concatenate(
        [res.results[c]["out"].reshape(BC, NSEQ, C) for c in range(8)], axis=0
    )
    return out.astype(np.float32)


# revision 7
# speedup vs baseline: 1.0026x; 1.0026x over previous
"""Trainium2 Bass kernel for ViT window attention with relative position bias.

Full inputs in, full outputs out. Data-parallel over batch: 64 batches split
8 per NeuronCore, weights replicated, no collectives.

Self-contained: hardcodes shapes and the (deterministic) relative-position
index structure; builds + compiles the Bass graph once per process.
"""

import os
import sys

for _p in ("/opt/trn_rl_repo", "/root/.axon_site/_ro/trn_rl_repo"):
    if os.path.isdir(_p) and _p not in sys.path:
        sys.path.insert(0, _p)

import numpy as np

import concourse.bass as bass
import concourse.mybir as mybir
import concourse.tile as tile
from concourse import bacc
from concourse.bass import AP
from concourse.masks import make_identity

F32 = mybir.dt.float32
BF16 = mybir.dt.bfloat16
AF = mybir.ActivationFunctionType

# problem constants
WIN = 14
NSEQ = WIN * WIN + 1          # 197
H = 16                        # heads
HD = 64                       # head dim
C = 1024
NREL = (2 * WIN - 1) * (2 * WIN - 1) + 3   # 732
B_FULL = 64
BC = 8                        # batches per core
T = BC * NSEQ                 # 1576 tokens per core
SCALE = HD ** -0.5            # 0.125
TCH = 394                     # qkv t-chunk (4 * 394 = 1576, fits one psum bank)
NT_TILE = 13                  # ceil(1576 / 128)
TPITCH = NT_TILE * 128        # 1664, xT free extent


def build_nc(sim: bool = False, stage: int = 99):
    nc = _build_graph(sim, stage)
    nc.compile()
    return nc


def _build_graph(sim: bool = False, stage: int = 99):
    if sim:
        nc = bacc.Bacc(None, target_bir_lowering=False, debug=True)
    else:
        nc = bacc.Bacc(None)

    def dump(nc, out_ext, ap, row0):
        nc.gpsimd.dma_start(out_ext[row0: row0 + ap.shape[0], 0: ap.free_size()], ap)

    x_ext = nc.declare_dram_parameter("x", [T, C], F32, isOutput=False)
    qkvw_ext = nc.declare_dram_parameter("qkv_w", [3 * C, C], F32, isOutput=False)
    qb_ext = nc.declare_dram_parameter("q_bias", [1, C], F32, isOutput=False)
    vb_ext = nc.declare_dram_parameter("v_bias", [1, C], F32, isOutput=False)
    rpb_ext = nc.declare_dram_parameter("rpb_table", [NREL, H], F32, isOutput=False)
    pw_ext = nc.declare_dram_parameter("proj_w", [C, C], F32, isOutput=False)
    pb_ext = nc.declare_dram_parameter("proj_b", [1, C], F32, isOutput=False)
    out_ext = nc.declare_dram_parameter("out", [T, C], F32, isOutput=True)

    # permutation constants: window stacks are loaded in j-major ascending
    # order from the shifted-table copies; these map load-rows to key order.
    sig_a = np.zeros(128, np.int64)
    for ki in range(9):
        for kj in range(14):
            # key row 1+14ki+kj needs off=364-27ki-kj = 135+27(8-ki)+(13-kj)
            sig_a[1 + 14 * ki + kj] = 1 + 9 * (13 - kj) + (8 - ki)
    sig_a[0] = 0
    sig_a[127] = 127
    sig_b = np.zeros(69, np.int64)
    for i in range(13):
        # key row i (kk=127+i) needs off=120-i = tabHb[4, 13+(12-i)... load
        # piece1 row j holds off 108+j -> j = 12 - i
        sig_b[i] = 12 - i
    for m in range(4):
        for kj in range(14):
            # key row 13+14m+kj needs off = 94-27m-kj = 27(3-m)+(13-kj)
            sig_b[13 + 14 * m + kj] = 13 + 4 * (13 - kj) + (3 - m)
    pa_np = np.zeros((128, 128), np.float32)
    pa_np[sig_a, np.arange(128)] = 1.0
    pb_np = np.zeros((69, 69), np.float32)
    pb_np[sig_b, np.arange(69)] = 1.0
    import ml_dtypes
    prev_a_d = nc.inline_tensor(pa_np.astype(ml_dtypes.bfloat16), name="prev_a")
    prev_b_d = nc.inline_tensor(pb_np.astype(ml_dtypes.bfloat16), name="prev_b")

    # DRAM staging tensors for the shifted table copies (exp'ed values)
    tabH_d = nc.dram_tensor("tabH_d", [H * 9, 379], BF16)
    tabHb_d = nc.dram_tensor("tabHb_d", [H * 5, 378], BF16)

    with tile.TileContext(nc) as tc:
        with tc.tile_pool(name="persist", bufs=1) as pp:
            ident_bf = pp.tile([128, 128], BF16, name="ident_bf", tag="ident_bf")
            make_identity(nc, ident_bf)
            ident_f = pp.tile([128, 128], F32, name="ident_f", tag="ident_f")
            make_identity(nc, ident_f)
            ones_row = pp.tile([1, 128], BF16, name="ones_row", tag="ones_row")
            nc.gpsimd.memset(ones_row[:], 1.0)
            onesK = pp.tile([128, HD], BF16, name="onesK", tag="onesK")
            nc.gpsimd.memset(onesK[:], 1.0)

            tabT = pp.tile([H, 736], BF16, name="tabT", tag="tabT")
            eb729 = pp.tile([128, H], BF16, name="eb729", tag="eb729")
            qb_col = pp.tile([128, 8], F32, name="qb_col", tag="qb_col")
            qb_row = pp.tile([1, C], F32, name="qb_row", tag="qb_row")
            vb_row = pp.tile([1, C], BF16, name="vb_row", tag="vb_row")
            pb_row = pp.tile([1, C], BF16, name="pb_row", tag="pb_row")
            vb_bcast = pp.tile([128, C], F32, name="vb_bcast", tag="vb_bcast")
            prev_a = pp.tile([128, 128], BF16, name="prev_a_s", tag="prev_a_s")
            prev_b = pp.tile([69, 69], BF16, name="prev_b_s", tag="prev_b_s")
            erow = []

            # persistent outputs of phase 1
            QKT = [
                pp.tile([128, T], BF16, name=f"qkt{ot}", tag=f"qkt{ot}")
                for ot in range(16)
            ]
            V1 = {}
            for b in range(BC):
                V1[(b, 0)] = pp.tile([128, H * HD], BF16,
                                     name=f"v1_{b}_0", tag=f"v1_{b}_0")
                V1[(b, 1)] = pp.tile([69, H * HD], BF16,
                                     name=f"v1_{b}_1", tag=f"v1_{b}_1")
            EBa = [pp.tile([128, NSEQ], BF16, name=f"eba{h}", tag=f"eba{h}")
                   for h in range(H)]
            EBb = [pp.tile([69, NSEQ], BF16, name=f"ebb{h}", tag=f"ebb{h}")
                   for h in range(H)]

            with (
                tc.tile_pool(name="xw", bufs=1) as xw,
                tc.tile_pool(name="stage", bufs=6) as stg,
                tc.tile_pool(name="cast", bufs=6) as cst,
                tc.tile_pool(name="wpad", bufs=8) as wpool,
                tc.tile_pool(name="ps_t", bufs=4, space="PSUM") as ps_t,
                tc.tile_pool(name="psum_mm", bufs=4, space="PSUM") as psmm,
            ):
                xT = [
                    xw.tile([128, TPITCH], BF16, name=f"xt{ct}", tag=f"xt{ct}")
                    for ct in range(8)
                ]
                WT = [
                    xw.tile([128, 3 * C], BF16, name=f"wt{ct}", tag=f"wt{ct}")
                    for ct in range(8)
                ]

                # ---- streaming load machinery -----------------------------
                # DMA queues, cast engines and copy-back engines round-robin
                # so no single engine serializes the load pipeline.
                dmaq = [nc.sync, nc.gpsimd, nc.scalar]
                ldq = [0]
                castq = [0]
                cbq = [0]

                def cast_eng():
                    e = (nc.vector, nc.scalar)[castq[0] % 2]
                    castq[0] += 1
                    return e

                def cb_copy(dst, src):
                    i = cbq[0] % 3
                    cbq[0] += 1
                    if i == 0:
                        nc.vector.tensor_copy(dst, src)
                    elif i == 1:
                        nc.scalar.copy(dst, src)
                    else:
                        nc.vector.tensor_copy(dst, src)

                def load_block(dst_list, src_ext, ob, dst_col):
                    rsz = min(128, src_ext.shape[0] - ob * 128)
                    for half in range(2):
                        sb = stg.tile([128, 512], F32, name="xstage",
                                      tag="xstage")
                        if rsz < 128:
                            nc.gpsimd.memset(sb[:], 0.0)
                        eng = dmaq[ldq[0] % 3]
                        ldq[0] += 1
                        eng.dma_start(
                            sb[0:rsz, :],
                            src_ext[ob * 128: ob * 128 + rsz,
                                    half * 512:(half + 1) * 512],
                        )
                        sbf = cst.tile([128, 512], BF16, name="bstage",
                                       tag="bstage")
                        ce = cast_eng()
                        if ce is nc.scalar:
                            ce.copy(sbf[:], sb[:])
                        else:
                            ce.tensor_copy(sbf[:], sb[:])
                        for ci in range(4):
                            ct = half * 4 + ci
                            ptr = ps_t.tile([128, 128], BF16, name="ptr",
                                            tag="ptr")
                            nc.tensor.transpose(
                                ptr[:], sbf[:, ci * 128:(ci + 1) * 128],
                                ident_bf[:]
                            )
                            dst = dst_list[ct][:, dst_col + ob * 128:
                                               dst_col + (ob + 1) * 128]
                            cb_copy(dst, ptr[:])

                # issue the first loads before anything else so DMA queues
                # fill from t=0 (first mini-chunk needs only x0 + W0)
                load_block(xT, x_ext, 0, 0)
                load_block(WT, qkvw_ext, 0, 0)
                for tt in range(1, 4):
                    load_block(xT, x_ext, tt, 0)
                load_block(WT, qkvw_ext, 1, 0)

                # ---- table prep (exp applied to the table ONCE, here) -----
                if True:
                    for j in range(6):
                        rs = min(128, NREL - j * 128)
                        tbj = pp.tile([128, H], F32, name="tbstage",
                                      tag="tbstage")
                        nc.gpsimd.dma_start(tbj[0:rs, :],
                                            rpb_ext[j * 128: j * 128 + rs, :])
                        ptp = ps_t.tile([H, 128], F32, name="tp", tag="ptr")
                        nc.tensor.transpose(ptp[:, 0:rs], tbj[0:rs, :],
                                            ident_f[0:rs, 0:rs])
                        nc.scalar.activation(
                            tabT[:, j * 128: j * 128 + rs], ptp[:, 0:rs],
                            AF.Exp,
                        )
                    nc.gpsimd.dma_start(prev_a[:], prev_a_d[:])
                    nc.gpsimd.dma_start(prev_b[:], prev_b_d[:])
                    # shifted copies: tabH_d[h*9+m, s] = tab[h, 135+27m+s],
                    #                 tabHb_d[h*5+m2, s] = tab[h, 27*m2+s]
                    tapc = tabT[:]
                    with nc.allow_non_contiguous_dma("shifted table copies"):
                        nc.gpsimd.dma_start(
                            tabH_d[:],
                            AP(tapc.tensor, tapc.offset + 135,
                               [[736, H], [27, 9], [1, 379]]),
                        )
                        nc.gpsimd.dma_start(
                            tabHb_d[:],
                            AP(tapc.tensor, tapc.offset,
                               [[736, H], [27, 5], [1, 378]]),
                        )

                    for j in range(3):
                        pte = ps_t.tile([1, H], BF16, name="te", tag="ptr")
                        nc.tensor.transpose(
                            pte[:], tabT[0:H, 729 + j: 730 + j],
                            ident_bf[0:H, 0:H]
                        )
                        er = pp.tile([1, H], BF16, name=f"erow{j}",
                                     tag=f"erow{j}")
                        nc.vector.tensor_copy(er[:], pte[:])
                        erow.append(er)
                    p729 = psmm.tile([128, H], F32, name="p729", tag="pq")
                    nc.tensor.matmul(p729[:], ones_row[:], erow[0][:],
                                     start=True, stop=True)
                    nc.scalar.copy(eb729[:], p729[:])

                    nc.sync.dma_start(qb_row[:], qb_ext[:])
                    nc.gpsimd.dma_start(vb_row[:], vb_ext[:])
                    nc.gpsimd.dma_start(pb_row[:], pb_ext[:])
                    # qb_col[p, ot] = q_bias[ot*128 + p] via 8 PE transposes
                    for ot in range(8):
                        ptq = ps_t.tile([128, 1], F32, name="tq", tag="ptr")
                        nc.tensor.transpose(
                            ptq[:], qb_row[0:1, ot * 128:(ot + 1) * 128],
                            ident_f[0:1, 0:1],
                        )
                        nc.vector.tensor_copy(qb_col[:, ot: ot + 1], ptq[:])
                    for oc in range(2):
                        pvb = psmm.tile([128, 512], F32, name="pbc", tag="pq")
                        nc.tensor.matmul(
                            pvb[:], ones_row[:],
                            vb_row[:, oc * 512:(oc + 1) * 512],
                            start=True, stop=True,
                        )
                        nc.scalar.copy(vb_bcast[:, oc * 512:(oc + 1) * 512],
                                       pvb[:])

                # --- interleaved EB machinery ------------------------------
                win_sets = []   # (wtmp, np_, prv, ebt, h, is_a)
                win_jobs = [(h, a) for h in range(H) for a in (True, False)]
                wdq = [0]

                def emit_window_set(job):
                    (h, is_a) = job
                    weng = (nc.gpsimd, nc.sync)[wdq[0] % 2]
                    wdq[0] += 1
                    wtmp = wpool.tile([128, 365], BF16, name="wtmp", tag="wtmp")
                    with nc.allow_non_contiguous_dma("toeplitz windows"):
                        if is_a:
                            # rows 1..126 j-major from tabH_d
                            weng.dma_start(
                                wtmp[1:127, :],
                                AP(tabH_d, h * 9 * 379,
                                   [[1, WIN], [379, 9], [1, 365]]),
                            )
                            # row 0 (dummy) + row 127 (off=121=tabHb[4,13])
                            wap = wtmp[:]
                            weng.dma_start(
                                AP(wap.tensor, wap.offset,
                                   [[365 * 127, 2], [1, 365]]),
                                AP(tabHb_d, (h * 5 + 4) * 378,
                                   [[13, 2], [1, 365]]),
                            )
                            np_, prv = 128, prev_a
                        else:
                            weng.dma_start(
                                wtmp[0:13, :],
                                AP(tabHb_d, (h * 5 + 4) * 378,
                                   [[1, 13], [1, 365]]),
                            )
                            weng.dma_start(
                                wtmp[13:69, :],
                                AP(tabHb_d, h * 5 * 378,
                                   [[1, WIN], [378, 4], [1, 365]]),
                            )
                            np_, prv = 69, prev_b
                    ebt = EBa[h] if is_a else EBb[h]
                    win_sets.append((wtmp, np_, prv, ebt, h, is_a))

                wrq = [0]

                def emit_wrev_extract(i):
                    (wtmp, np_, prv, ebt, h, is_a) = win_sets[i]
                    wrev = psmm.tile([128, 365], F32, name="wrev", tag="pq")
                    nc.tensor.matmul(
                        wrev[0:np_, :], prv[0:np_, 0:np_], wtmp[0:np_, :],
                        start=True, stop=True,
                    )
                    wap = wrev[0:np_, :]
                    ebtap = ebt[:]
                    # table already exp'ed: extraction is a plain copy
                    xeng = (nc.vector, nc.scalar)[wrq[0] % 2]
                    wrq[0] += 1
                    dst_ap = AP(ebtap.tensor, ebtap.offset + 1,
                                [[NSEQ, np_], [WIN, WIN], [1, WIN]])
                    src_ap = AP(wap.tensor, wap.offset,
                                [[365, np_], [27, WIN], [1, WIN]])
                    if xeng is nc.scalar:
                        xeng.copy(dst_ap, src_ap)
                    else:
                        xeng.tensor_copy(dst_ap, src_ap)
                    nc.vector.tensor_copy(
                        ebt[0:np_, 0:1], eb729[0:np_, h: h + 1]
                    )
                    if not is_a:  # after the b-tile, patch the CLS key row
                        e1 = erow[1][:]
                        nc.vector.tensor_copy(
                            EBa[h][0:1, 1:NSEQ],
                            AP(e1.tensor, e1.offset + h,
                               [[H, 1], [0, NSEQ - 1]]),
                        )
                        nc.vector.tensor_copy(
                            EBa[h][0:1, 0:1], erow[2][0:1, h: h + 1]
                        )

                wctr = [0]
                wrctr = [0]

                def b1_span(ot, t0, tsz):
                    pq = psmm.tile([128, 512], F32, name="pq", tag="pq")
                    for ct in range(8):
                        nc.tensor.matmul(
                            pq[:, 0:tsz],
                            WT[ct][:, ot * 128:(ot + 1) * 128],
                            xT[ct][:, t0: t0 + tsz],
                            start=(ct == 0), stop=(ct == 7),
                        )
                    if ot < 8:
                        nc.vector.tensor_scalar_add(
                            QKT[ot][:, t0: t0 + tsz], pq[:, 0:tsz],
                            qb_col[:, ot: ot + 1],
                        )
                    else:
                        nc.vector.tensor_copy(
                            QKT[ot][:, t0: t0 + tsz], pq[:, 0:tsz]
                        )

                def b1_chunk(ot, tci):
                    b1_span(ot, tci * TCH, TCH)
                    if wctr[0] < len(win_jobs):
                        emit_window_set(win_jobs[wctr[0]])
                        wctr[0] += 1
                    if wrctr[0] + 8 < wctr[0] and wrctr[0] < len(win_jobs):
                        emit_wrev_extract(wrctr[0])
                        wrctr[0] += 1

                def b1_otile(ot):
                    for tci in range(4):
                        b1_chunk(ot, tci)

                # x-tiles needed per 394-token chunk of B1(ot=0):
                # chunk 0 -> x0..3, 1 -> x4..6(+3), 2 -> x7..9, 3 -> x10..12
                # first chunk split so PE can start after just x0 + W0
                b1_span(0, 0, 128)
                b1_span(0, 128, 266)
                for ci, xts in enumerate(((4, 5, 6), (7, 8, 9), (10, 11, 12))):
                    for tt in xts:
                        load_block(xT, x_ext, tt, 0)
                    b1_chunk(0, ci + 1)
                b1_chunk(1, 0)
                b1_chunk(1, 1)
                for ob in range(2, 24):
                    load_block(WT, qkvw_ext, ob, 0)
                    if ob < 16:
                        b1_otile(ob)
                b1_chunk(1, 2)
                b1_chunk(1, 3)
                while wctr[0] < len(win_jobs):
                    emit_window_set(win_jobs[wctr[0]])
                    wctr[0] += 1

                if stage <= 2:
                    dump(nc, out_ext, xT[0][:, 0:1024], 0)
                    dump(nc, out_ext, WT[0][:, 0:1024], 128)
                    return nc

                # V projection -> V1[(b,kt)] bf16 (+bias)
                for b in range(BC):
                    for kt, (ko, ksz) in enumerate(((0, 128), (128, 69))):
                        t0 = b * NSEQ + ko
                        v1t = V1[(b, kt)]
                        for oc in range(2):
                            pv = psmm.tile([128, 512], F32, name="pv", tag="pq")
                            for ct in range(8):
                                nc.tensor.matmul(
                                    pv[0:ksz, :],
                                    xT[ct][:, t0: t0 + ksz],
                                    WT[ct][:, 2 * C + oc * 512:
                                            2 * C + (oc + 1) * 512],
                                    start=(ct == 0), stop=(ct == 7),
                                )
                            nc.vector.tensor_add(
                                v1t[0:ksz, oc * 512:(oc + 1) * 512],
                                pv[0:ksz, :],
                                vb_bcast[0:ksz, oc * 512:(oc + 1) * 512],
                            )
                        if wrctr[0] < len(win_sets):
                            emit_wrev_extract(wrctr[0])
                            wrctr[0] += 1
                        if wrctr[0] < len(win_sets):
                            emit_wrev_extract(wrctr[0])
                            wrctr[0] += 1
                while wrctr[0] < len(win_sets):
                    emit_wrev_extract(wrctr[0])
                    wrctr[0] += 1

            if stage <= 1 or stage == 4:
                for hh in range(4):
                    dump(nc, out_ext, EBa[hh][:, :], hh * 128)
                return nc

            # ----- phase 2: attention + output projection -------------------
            from contextlib import ExitStack
            with (
                tc.tile_pool(name="attbig", bufs=1) as ab,
                tc.tile_pool(name="stage2", bufs=6) as stage2,
                tc.tile_pool(name="stage2b", bufs=6) as stage2b,
                tc.tile_pool(name="attw", bufs=8) as attw,
                tc.tile_pool(name="attsmall", bufs=6) as attsmall,
                tc.tile_pool(name="ostage", bufs=3) as op_,
            ):
                att_ctx = ExitStack()
                ps_s = att_ctx.enter_context(
                    tc.tile_pool(name="ps_s", bufs=4, space="PSUM"))
                ps_o = att_ctx.enter_context(
                    tc.tile_pool(name="ps_o", bufs=2, space="PSUM"))
                ps_r = att_ctx.enter_context(
                    tc.tile_pool(name="ps_r", bufs=2, space="PSUM"))
                ps_j = ps_r
                attnT = [
                    ab.tile([128, T], BF16, name=f"at{ct}", tag=f"at{ct}")
                    for ct in range(8)
                ]
                PWT = [
                    ab.tile([128, C], BF16, name=f"pwt{ct}", tag=f"pwt{ct}")
                    for ct in range(8)
                ]
                pb_bcast = ab.tile([128, C], F32, name="pb_bcast", tag="pb_bcast")
                for oc in range(2):
                    ppb = ps_r.tile([128, 512], F32, name="pbc2", tag="rbp")
                    nc.tensor.matmul(
                        ppb[:], ones_row[:], pb_row[:, oc * 512:(oc + 1) * 512],
                        start=True, stop=True,
                    )
                    nc.scalar.copy(pb_bcast[:, oc * 512:(oc + 1) * 512], ppb[:])

                pw_jobs = []
                pwdq = [0]
                for ob in range(8):
                    for half in range(2):
                        pwb = stage2.tile([128, 512], F32, name="pwstage",
                                          tag="pwstage")
                        peng = dmaq[pwdq[0] % 3]
                        pwdq[0] += 1
                        peng.dma_start(
                            pwb[:],
                            pw_ext[ob * 128:(ob + 1) * 128,
                                   half * 512:(half + 1) * 512])
                        pwbf = stage2b.tile([128, 512], BF16, name="pwbstage",
                                            tag="pwbstage")
                        nc.vector.tensor_copy(pwbf[:], pwb[:])
                        for ci in range(4):
                            pw_jobs.append((pwbf, ob, half * 4 + ci))
                # stage2 pool must keep pwbf tiles alive until their
                # transposes are emitted (spread through early attention)
                pwctr = [0]

                def emit_pw_transposes(n):
                    while n > 0 and pwctr[0] < len(pw_jobs):
                        (pwbf, ob, ct) = pw_jobs[pwctr[0]]
                        ci = ct % 4
                        pwctr[0] += 1
                        n -= 1
                        ptr = ps_r.tile([128, 128], BF16, name="ptr2", tag="rbp")
                        nc.tensor.transpose(
                            ptr[:], pwbf[:, ci * 128:(ci + 1) * 128], ident_bf[:]
                        )
                        dst = PWT[ct][:, ob * 128:(ob + 1) * 128]
                        if (ob + ct) % 2 == 0:
                            nc.scalar.copy(dst, ptr[:])
                        else:
                            nc.vector.tensor_copy(dst, ptr[:])

                emit_pw_transposes(24)

                b4q = []          # pending out-proj tiles
                b4cur = [None]    # (tt, ost, next_oc)

                def b4_tick():
                    # emit half an out-proj tile (8 matmuls) as PE filler
                    if b4cur[0] is None:
                        if not b4q:
                            return False
                        tt = b4q.pop(0)
                        ost = op_.tile([128, C], F32, name="ost", tag="ost")
                        b4cur[0] = (tt, ost, 0)
                    (tt, ost, oc) = b4cur[0]
                    tsz = min(128, T - tt * 128)
                    pp2 = ps_j.tile([128, 512], F32, name="ppj", tag="rbp")
                    for ct in range(8):
                        nc.tensor.matmul(
                            pp2[0:tsz, :],
                            attnT[ct][:, tt * 128: tt * 128 + tsz],
                            PWT[ct][:, oc * 512:(oc + 1) * 512],
                            start=(ct == 0), stop=(ct == 7),
                        )
                    nc.vector.tensor_add(
                        ost[0:tsz, oc * 512:(oc + 1) * 512],
                        pp2[0:tsz, :],
                        pb_bcast[0:tsz, oc * 512:(oc + 1) * 512],
                    )
                    if oc == 0:
                        b4cur[0] = (tt, ost, 1)
                    else:
                        nc.sync.dma_start(
                            out_ext[tt * 128: tt * 128 + tsz, :], ost[0:tsz, :]
                        )
                        b4cur[0] = None
                    return True

                b4ctr = [0]
                # attention: head-paired column-tiled PV/normalize.
                # All 8 QK matmuls of a block are emitted before the first
                # rowsum so the in-order PE queue never stalls on the
                # exp->mul chain; a half out-proj tile fills the gap.
                for bp in range(BC // 2):
                    t0p = 2 * bp * NSEQ
                    for hp in range(8):
                        po = ps_o.tile([128, 2 * NSEQ], F32, name="po",
                                       tag="po", padded_shape=[128, 512])
                        rbp = ps_r.tile([128, 2 * NSEQ], F32, name="rbp",
                                        tag="rbp", padded_shape=[128, 512])
                        kot = 8 + hp
                        ptd = {}
                        for hh in range(2):
                            h = 2 * hp + hh
                            qpo = 64 * hh
                            for kt, (ko, ksz) in enumerate(((0, 128),
                                                            (128, 69))):
                                ps = ps_s.tile([128, 2 * NSEQ], F32,
                                               name="ps", tag="ps")
                                for bi in range(2):
                                    b = 2 * bp + bi
                                    t0 = b * NSEQ
                                    nc.tensor.matmul(
                                        ps[0:ksz, bi * NSEQ:(bi + 1) * NSEQ],
                                        QKT[kot][qpo: qpo + 64,
                                                 t0 + ko: t0 + ko + ksz],
                                        QKT[hp][qpo: qpo + 64, t0: t0 + NSEQ],
                                        start=True, stop=True,
                                    )
                                pt = attw.tile([128, 2 * NSEQ], BF16,
                                               name="pt", tag="pt")
                                nc.scalar.activation(
                                    pt[0:ksz, :], ps[0:ksz, :], AF.Exp,
                                    scale=SCALE,
                                )
                                eb = (EBa[h] if kt == 0 else EBb[h])
                                for (meng, bi2) in ((nc.vector, 0),
                                                    (nc.gpsimd, 1)):
                                    meng.tensor_mul(
                                        pt[0:ksz, bi2 * NSEQ:(bi2 + 1) * NSEQ],
                                        pt[0:ksz, bi2 * NSEQ:(bi2 + 1) * NSEQ],
                                        eb[0:ksz, :],
                                    )
                                ptd[(hh, kt)] = (pt, ksz)
                        # independent PE work while the exp/mul chains run
                        if not b4_tick():
                            emit_pw_transposes(8)
                        for hh in range(2):
                            h = 2 * hp + hh
                            cpos = 64 * hh
                            for kt in range(2):
                                pt, ksz = ptd[(hh, kt)]
                                nc.tensor.matmul(
                                    rbp[cpos: cpos + 64, :],
                                    onesK[0:ksz, :], pt[0:ksz, :],
                                    start=(kt == 0), stop=(kt == 1),
                                    tile_position=(0, cpos),
                                )
                            for bi in range(2):
                                b = 2 * bp + bi
                                for kt, (ko, ksz) in enumerate(((0, 128),
                                                                (128, 69))):
                                    nc.tensor.matmul(
                                        po[cpos: cpos + 64,
                                           bi * NSEQ:(bi + 1) * NSEQ],
                                        V1[(b, kt)][0:ksz,
                                                    h * HD:(h + 1) * HD],
                                        ptd[(hh, kt)][0][0:ksz,
                                                   bi * NSEQ:(bi + 1) * NSEQ],
                                        start=(kt == 0), stop=(kt == 1),
                                        tile_position=(0, cpos),
                                    )
                        rbs = attsmall.tile([128, 2 * NSEQ], F32,
                                            name="rbs", tag="rbs")
                        nc.vector.reciprocal_approx_fast(rbs[:], rbp[:])
                        nc.vector.tensor_mul(
                            attnT[hp][:, t0p: t0p + 2 * NSEQ],
                            po[:],
                            rbs[:],
                        )
                    while (b4ctr[0] + 1) * 128 <= (bp + 1) * 2 * NSEQ:
                        b4q.append(b4ctr[0])
                        b4ctr[0] += 1

                if stage <= 5:
                    dump(nc, out_ext, attnT[0][:, 0:1024], 0)
                    return nc

                # flush remaining output-projection tiles
                while b4ctr[0] < NT_TILE:
                    b4q.append(b4ctr[0])
                    b4ctr[0] += 1
                while b4q or b4cur[0] is not None:
                    b4_tick()
                att_ctx.close()

    return nc


_NC = None
LAST_RESULT = None


def _get_nc():
    global _NC
    if _NC is None:
        _NC = build_nc()
    return _NC


def make_in_maps(x, qkv_w, q_bias, v_bias, rpb_table, proj_w, proj_b):
    x = np.ascontiguousarray(np.asarray(x, np.float32))
    qkv_w = np.ascontiguousarray(np.asarray(qkv_w, np.float32))
    q_bias = np.ascontiguousarray(np.asarray(q_bias, np.float32).reshape(1, C))
    v_bias = np.ascontiguousarray(np.asarray(v_bias, np.float32).reshape(1, C))
    rpb_table = np.ascontiguousarray(np.asarray(rpb_table, np.float32))
    proj_w = np.ascontiguousarray(np.asarray(proj_w, np.float32))
    proj_b = np.ascontiguousarray(np.asarray(proj_b, np.float32).reshape(1, C))
    in_maps = []
    for c in range(8):
        xs = np.ascontiguousarray(
            x[c * BC:(c + 1) * BC].reshape(T, C)
        )
        in_maps.append({
            "x": xs, "qkv_w": qkv_w, "q_bias": q_bias, "v_bias": v_bias,
            "rpb_table": rpb_table, "proj_w": proj_w, "proj_b": proj_b,
        })
    return in_maps


def _ensure_axon_hooks_importable():
    """bass_utils imports antenv.axon_hooks when BASS_TRACE is set; the image's
    antenv lacks that module. Provide a no-op stand-in so tracing degrades
    gracefully instead of crashing (unless a real one is already installed)."""
    import types
    try:
        import antenv.axon_hooks  # noqa: F401
    except Exception:
        mod = types.ModuleType("antenv.axon_hooks")
        mod._h = None
        mod.set_axon_ntff_profile_hook = lambda h: setattr(mod, "_h", h)
        mod.get_axon_ntff_profile_hook = lambda: mod._h
        sys.modules["antenv.axon_hooks"] = mod
        try:
            import antenv
            antenv.axon_hooks = mod
        except Exception:
            pass


def kernel(x, qkv_w, q_bias, v_bias, rpb_table, proj_w, proj_b,
           rel_pos_index=None, **_unused):
    global LAST_RESULT
    _ensure_axon_hooks_importable()
    from concourse.bass_utils import run_bass_kernel_spmd

    nc = _get_nc()
    in_maps = make_in_maps(x, qkv_w, q_bias, v_bias, rpb_table, proj_w, proj_b)
    res = run_bass_kernel_spmd(nc, in_maps, core_ids=list(range(8)))
    LAST_RESULT = res
    out = np.concatenate(
        [res.results[c]["out"].reshape(BC, NSEQ, C) for c in range(8)], axis=0
    )
    return out.astype(np.float32)


# revision 9
# speedup vs baseline: 1.1918x; 1.1887x over previous
"""Trainium2 Bass kernel for ViT window attention with relative position bias.

Full inputs in, full outputs out. Data-parallel over batch: 64 batches split
8 per NeuronCore, weights replicated, no collectives.

Self-contained: hardcodes shapes and the (deterministic) relative-position
index structure; builds + compiles the Bass graph once per process.
"""

import os
import sys

for _p in ("/opt/trn_rl_repo", "/root/.axon_site/_ro/trn_rl_repo"):
    if os.path.isdir(_p) and _p not in sys.path:
        sys.path.insert(0, _p)

import numpy as np

import concourse.bass as bass
import concourse.mybir as mybir
import concourse.tile as tile
from concourse import bacc
from concourse.bass import AP
from concourse.masks import make_identity

F32 = mybir.dt.float32
BF16 = mybir.dt.bfloat16
AF = mybir.ActivationFunctionType

# problem constants
WIN = 14
NSEQ = WIN * WIN + 1          # 197
H = 16                        # heads
HD = 64                       # head dim
C = 1024
NREL = (2 * WIN - 1) * (2 * WIN - 1) + 3   # 732
B_FULL = 64
BC = 8                        # batches per core
T = BC * NSEQ                 # 1576 tokens per core
SCALE = HD ** -0.5            # 0.125
TCH = 394                     # qkv t-chunk (4 * 394 = 1576, fits one psum bank)
NT_TILE = 13                  # ceil(1576 / 128)
TPITCH = NT_TILE * 128        # 1664, xT free extent


def build_nc(sim: bool = False, stage: int = 99):
    nc = _build_graph(sim, stage)
    nc.compile()
    return nc


def _build_graph(sim: bool = False, stage: int = 99):
    if sim:
        nc = bacc.Bacc(None, target_bir_lowering=False, debug=True)
    else:
        nc = bacc.Bacc(None)

    def dump(nc, out_ext, ap, row0):
        nc.gpsimd.dma_start(out_ext[row0: row0 + ap.shape[0], 0: ap.free_size()], ap)

    x_ext = nc.declare_dram_parameter("x", [T, C], F32, isOutput=False)
    qkvw_ext = nc.declare_dram_parameter("qkv_w", [3 * C, C], F32, isOutput=False)
    qb_ext = nc.declare_dram_parameter("q_bias", [1, C], F32, isOutput=False)
    vb_ext = nc.declare_dram_parameter("v_bias", [1, C], F32, isOutput=False)
    rpb_ext = nc.declare_dram_parameter("rpb_table", [NREL, H], F32, isOutput=False)
    pw_ext = nc.declare_dram_parameter("proj_w", [C, C], F32, isOutput=False)
    pb_ext = nc.declare_dram_parameter("proj_b", [1, C], F32, isOutput=False)
    out_ext = nc.declare_dram_parameter("out", [T, C], F32, isOutput=True)

    # permutation constants: window stacks are loaded in j-major ascending
    # order from the shifted-table copies; these map load-rows to key order.
    sig_a = np.zeros(128, np.int64)
    for ki in range(9):
        for kj in range(14):
            # key row 1+14ki+kj needs off=364-27ki-kj = 135+27(8-ki)+(13-kj)
            sig_a[1 + 14 * ki + kj] = 1 + 9 * (13 - kj) + (8 - ki)
    sig_a[0] = 0
    sig_a[127] = 127
    sig_b = np.zeros(69, np.int64)
    for i in range(13):
        # key row i (kk=127+i) needs off=120-i = tabHb[4, 13+(12-i)... load
        # piece1 row j holds off 108+j -> j = 12 - i
        sig_b[i] = 12 - i
    for m in range(4):
        for kj in range(14):
            # key row 13+14m+kj needs off = 94-27m-kj = 27(3-m)+(13-kj)
            sig_b[13 + 14 * m + kj] = 13 + 4 * (13 - kj) + (3 - m)
    pa_np = np.zeros((128, 128), np.float32)
    pa_np[sig_a, np.arange(128)] = 1.0
    pb_np = np.zeros((69, 69), np.float32)
    pb_np[sig_b, np.arange(69)] = 1.0
    import ml_dtypes
    prev_a_d = nc.inline_tensor(pa_np.astype(ml_dtypes.bfloat16), name="prev_a")
    prev_b_d = nc.inline_tensor(pb_np.astype(ml_dtypes.bfloat16), name="prev_b")

    # DRAM staging tensors for the shifted table copies (exp'ed values)
    tabH_d = nc.dram_tensor("tabH_d", [H * 9, 379], BF16)
    tabHb_d = nc.dram_tensor("tabHb_d", [H * 5, 378], BF16)

    with tile.TileContext(nc) as tc:
        with tc.tile_pool(name="persist", bufs=1) as pp:
            ident_bf = pp.tile([128, 128], BF16, name="ident_bf", tag="ident_bf")
            make_identity(nc, ident_bf)
            ident_f = pp.tile([128, 128], F32, name="ident_f", tag="ident_f")
            make_identity(nc, ident_f)
            ones_row = pp.tile([1, 128], BF16, name="ones_row", tag="ones_row")
            nc.gpsimd.memset(ones_row[:], 1.0)
            onesK = pp.tile([128, HD], BF16, name="onesK", tag="onesK")
            nc.gpsimd.memset(onesK[:], 1.0)

            tabT = pp.tile([H, 736], BF16, name="tabT", tag="tabT")
            eb729 = pp.tile([128, H], BF16, name="eb729", tag="eb729")
            qb_col = pp.tile([128, 8], F32, name="qb_col", tag="qb_col")
            qb_row = pp.tile([1, C], F32, name="qb_row", tag="qb_row")
            vb_row = pp.tile([1, C], BF16, name="vb_row", tag="vb_row")
            pb_row = pp.tile([1, C], BF16, name="pb_row", tag="pb_row")
            vb_bcast = pp.tile([128, C], F32, name="vb_bcast", tag="vb_bcast")
            prev_a = pp.tile([128, 128], BF16, name="prev_a_s", tag="prev_a_s")
            prev_b = pp.tile([69, 69], BF16, name="prev_b_s", tag="prev_b_s")
            erow = []

            # persistent outputs of phase 1
            QKT = [
                pp.tile([128, T], BF16, name=f"qkt{ot}", tag=f"qkt{ot}")
                for ot in range(16)
            ]
            V1 = {}
            for b in range(BC):
                V1[(b, 0)] = pp.tile([128, H * HD], BF16,
                                     name=f"v1_{b}_0", tag=f"v1_{b}_0")
                V1[(b, 1)] = pp.tile([69, H * HD], BF16,
                                     name=f"v1_{b}_1", tag=f"v1_{b}_1")
            EBa = [pp.tile([128, NSEQ], BF16, name=f"eba{h}", tag=f"eba{h}")
                   for h in range(H)]
            EBb = [pp.tile([69, NSEQ], BF16, name=f"ebb{h}", tag=f"ebb{h}")
                   for h in range(H)]

            with (
                tc.tile_pool(name="xw", bufs=1) as xw,
                tc.tile_pool(name="stage", bufs=5) as stg,
                tc.tile_pool(name="cast", bufs=8) as cst,
                tc.tile_pool(name="wpad", bufs=8) as wpool,
                tc.tile_pool(name="ps_t", bufs=4, space="PSUM") as ps_t,
                tc.tile_pool(name="psum_mm", bufs=4, space="PSUM") as psmm,
            ):
                xT = [
                    xw.tile([128, TPITCH], BF16, name=f"xt{ct}", tag=f"xt{ct}")
                    for ct in range(8)
                ]
                WT = [
                    xw.tile([128, 3 * C], BF16, name=f"wt{ct}", tag=f"wt{ct}")
                    for ct in range(8)
                ]

                # ---- streaming load machinery -----------------------------
                # DMA queues, cast engines and copy-back engines round-robin
                # so no single engine serializes the load pipeline.
                dmaq = [nc.sync, nc.gpsimd, nc.scalar]
                ldq = [0]
                castq = [0]
                cbq = [0]

                def cast_eng():
                    e = (nc.vector, nc.scalar)[castq[0] % 2]
                    castq[0] += 1
                    return e

                def cb_copy(dst, src):
                    i = cbq[0] % 3
                    cbq[0] += 1
                    if i == 0:
                        nc.vector.tensor_copy(dst, src)
                    elif i == 1:
                        nc.scalar.copy(dst, src)
                    else:
                        nc.vector.tensor_copy(dst, src)

                # loads are DMA+cast only; PE transposes are flushed
                # lazily so ready b1 matmuls never queue behind transposes
                # whose casts haven't landed yet.
                tp_pending = []   # (sbf, dst_list, col0, ct_base, key)
                tp_done = set()   # keys whose transposes are all emitted

                def load_block(dst_list, src_ext, ob, dst_col):
                    rsz = min(128, src_ext.shape[0] - ob * 128)
                    for half in range(2):
                        sb = stg.tile([128, 512], F32, name="xstage",
                                      tag="xstage")
                        if rsz < 128:
                            nc.gpsimd.memset(sb[:], 0.0)
                        eng = dmaq[ldq[0] % 3]
                        ldq[0] += 1
                        eng.dma_start(
                            sb[0:rsz, :],
                            src_ext[ob * 128: ob * 128 + rsz,
                                    half * 512:(half + 1) * 512],
                        )
                        sbf = cst.tile([128, 512], BF16, name="bstage",
                                       tag="bstage")
                        ce = cast_eng()
                        if ce is nc.scalar:
                            ce.copy(sbf[:], sb[:])
                        else:
                            ce.tensor_copy(sbf[:], sb[:])
                        tp_pending.append(
                            (sbf, dst_list, dst_col + ob * 128, half * 4,
                             (id(dst_list), ob, half)))

                def flush_half(n=1):
                    while n > 0 and tp_pending:
                        (sbf, dst_list, col0, ct_base, key) = tp_pending.pop(0)
                        for ci in range(4):
                            ct = ct_base + ci
                            ptr = ps_t.tile([128, 128], BF16, name="ptr",
                                            tag="ptr")
                            nc.tensor.transpose(
                                ptr[:], sbf[:, ci * 128:(ci + 1) * 128],
                                ident_bf[:]
                            )
                            cb_copy(dst_list[ct][:, col0: col0 + 128], ptr[:])
                        tp_done.add(key)
                        n -= 1

                def flush_until(dst_list, ob):
                    while ((id(dst_list), ob, 1) not in tp_done
                           and tp_pending):
                        flush_half(1)

                # issue the first loads before anything else so DMA queues
                # fill from t=0 (first mini-chunk needs only x0 + W0)
                load_block(xT, x_ext, 0, 0)
                load_block(WT, qkvw_ext, 0, 0)
                for tt in range(1, 4):
                    load_block(xT, x_ext, tt, 0)
                load_block(WT, qkvw_ext, 1, 0)

                # ---- table prep (exp applied to the table ONCE, here) -----
                if True:
                    for j in range(6):
                        rs = min(128, NREL - j * 128)
                        tbj = pp.tile([128, H], F32, name="tbstage",
                                      tag="tbstage")
                        nc.gpsimd.dma_start(tbj[0:rs, :],
                                            rpb_ext[j * 128: j * 128 + rs, :])
                        ptp = ps_t.tile([H, 128], F32, name="tp", tag="ptr")
                        nc.tensor.transpose(ptp[:, 0:rs], tbj[0:rs, :],
                                            ident_f[0:rs, 0:rs])
                        nc.scalar.activation(
                            tabT[:, j * 128: j * 128 + rs], ptp[:, 0:rs],
                            AF.Exp,
                        )
                    nc.gpsimd.dma_start(prev_a[:], prev_a_d[:])
                    nc.gpsimd.dma_start(prev_b[:], prev_b_d[:])
                    # shifted copies: tabH_d[h*9+m, s] = tab[h, 135+27m+s],
                    #                 tabHb_d[h*5+m2, s] = tab[h, 27*m2+s]
                    tapc = tabT[:]
                    with nc.allow_non_contiguous_dma("shifted table copies"):
                        nc.gpsimd.dma_start(
                            tabH_d[:],
                            AP(tapc.tensor, tapc.offset + 135,
                               [[736, H], [27, 9], [1, 379]]),
                        )
                        nc.gpsimd.dma_start(
                            tabHb_d[:],
                            AP(tapc.tensor, tapc.offset,
                               [[736, H], [27, 5], [1, 378]]),
                        )

                    for j in range(3):
                        pte = ps_t.tile([1, H], BF16, name="te", tag="ptr")
                        nc.tensor.transpose(
                            pte[:], tabT[0:H, 729 + j: 730 + j],
                            ident_bf[0:H, 0:H]
                        )
                        er = pp.tile([1, H], BF16, name=f"erow{j}",
                                     tag=f"erow{j}")
                        nc.vector.tensor_copy(er[:], pte[:])
                        erow.append(er)
                    p729 = psmm.tile([128, H], F32, name="p729", tag="pq")
                    nc.tensor.matmul(p729[:], ones_row[:], erow[0][:],
                                     start=True, stop=True)
                    nc.scalar.copy(eb729[:], p729[:])

                    nc.sync.dma_start(qb_row[:], qb_ext[:])
                    nc.gpsimd.dma_start(vb_row[:], vb_ext[:])
                    nc.gpsimd.dma_start(pb_row[:], pb_ext[:])
                    # qb_col[p, ot] = q_bias[ot*128 + p] via 8 PE transposes
                    for ot in range(8):
                        ptq = ps_t.tile([128, 1], F32, name="tq", tag="ptr")
                        nc.tensor.transpose(
                            ptq[:], qb_row[0:1, ot * 128:(ot + 1) * 128],
                            ident_f[0:1, 0:1],
                        )
                        nc.vector.tensor_copy(qb_col[:, ot: ot + 1], ptq[:])
                    for oc in range(2):
                        pvb = psmm.tile([128, 512], F32, name="pbc", tag="pq")
                        nc.tensor.matmul(
                            pvb[:], ones_row[:],
                            vb_row[:, oc * 512:(oc + 1) * 512],
                            start=True, stop=True,
                        )
                        nc.scalar.copy(vb_bcast[:, oc * 512:(oc + 1) * 512],
                                       pvb[:])

                # --- interleaved EB machinery ------------------------------
                win_sets = []   # (wtmp, np_, prv, ebt, h, is_a)
                win_jobs = [(h, a) for h in range(H) for a in (True, False)]
                wdq = [0]

                def emit_window_set(job):
                    (h, is_a) = job
                    weng = (nc.gpsimd, nc.sync)[wdq[0] % 2]
                    wdq[0] += 1
                    wtmp = wpool.tile([128, 365], BF16, name="wtmp", tag="wtmp")
                    with nc.allow_non_contiguous_dma("toeplitz windows"):
                        if is_a:
                            # rows 1..126 j-major from tabH_d
                            weng.dma_start(
                                wtmp[1:127, :],
                                AP(tabH_d, h * 9 * 379,
                                   [[1, WIN], [379, 9], [1, 365]]),
                            )
                            # row 0 (dummy) + row 127 (off=121=tabHb[4,13])
                            wap = wtmp[:]
                            weng.dma_start(
                                AP(wap.tensor, wap.offset,
                                   [[365 * 127, 2], [1, 365]]),
                                AP(tabHb_d, (h * 5 + 4) * 378,
                                   [[13, 2], [1, 365]]),
                            )
                            np_, prv = 128, prev_a
                        else:
                            weng.dma_start(
                                wtmp[0:13, :],
                                AP(tabHb_d, (h * 5 + 4) * 378,
                                   [[1, 13], [1, 365]]),
                            )
                            weng.dma_start(
                                wtmp[13:69, :],
                                AP(tabHb_d, h * 5 * 378,
                                   [[1, WIN], [378, 4], [1, 365]]),
                            )
                            np_, prv = 69, prev_b
                    ebt = EBa[h] if is_a else EBb[h]
                    win_sets.append((wtmp, np_, prv, ebt, h, is_a))

                wrq = [0]

                def emit_wrev_extract(i):
                    (wtmp, np_, prv, ebt, h, is_a) = win_sets[i]
                    wrev = psmm.tile([128, 365], F32, name="wrev", tag="pq")
                    nc.tensor.matmul(
                        wrev[0:np_, :], prv[0:np_, 0:np_], wtmp[0:np_, :],
                        start=True, stop=True,
                    )
                    wap = wrev[0:np_, :]
                    ebtap = ebt[:]
                    # table already exp'ed: extraction is a plain copy
                    xeng = (nc.vector, nc.scalar)[wrq[0] % 2]
                    wrq[0] += 1
                    dst_ap = AP(ebtap.tensor, ebtap.offset + 1,
                                [[NSEQ, np_], [WIN, WIN], [1, WIN]])
                    src_ap = AP(wap.tensor, wap.offset,
                                [[365, np_], [27, WIN], [1, WIN]])
                    if xeng is nc.scalar:
                        xeng.copy(dst_ap, src_ap)
                    else:
                        xeng.tensor_copy(dst_ap, src_ap)
                    nc.vector.tensor_copy(
                        ebt[0:np_, 0:1], eb729[0:np_, h: h + 1]
                    )
                    if not is_a:  # after the b-tile, patch the CLS key row
                        e1 = erow[1][:]
                        nc.vector.tensor_copy(
                            EBa[h][0:1, 1:NSEQ],
                            AP(e1.tensor, e1.offset + h,
                               [[H, 1], [0, NSEQ - 1]]),
                        )
                        nc.vector.tensor_copy(
                            EBa[h][0:1, 0:1], erow[2][0:1, h: h + 1]
                        )

                wctr = [0]
                wrctr = [0]

                def b1_span(ot, t0, tsz):
                    pq = psmm.tile([128, 512], F32, name="pq", tag="pq")
                    for ct in range(8):
                        nc.tensor.matmul(
                            pq[:, 0:tsz],
                            WT[ct][:, ot * 128:(ot + 1) * 128],
                            xT[ct][:, t0: t0 + tsz],
                            start=(ct == 0), stop=(ct == 7),
                        )
                    if ot < 8:
                        nc.vector.tensor_scalar_add(
                            QKT[ot][:, t0: t0 + tsz], pq[:, 0:tsz],
                            qb_col[:, ot: ot + 1],
                        )
                    else:
                        nc.vector.tensor_copy(
                            QKT[ot][:, t0: t0 + tsz], pq[:, 0:tsz]
                        )

                def b1_chunk(ot, tci):
                    b1_span(ot, tci * TCH, TCH)
                    if wctr[0] < len(win_jobs):
                        emit_window_set(win_jobs[wctr[0]])
                        wctr[0] += 1
                    if wrctr[0] + 8 < wctr[0] and wrctr[0] < len(win_jobs):
                        emit_wrev_extract(wrctr[0])
                        wrctr[0] += 1

                def b1_otile(ot):
                    for tci in range(4):
                        b1_chunk(ot, tci)

                # x-tiles needed per 394-token chunk of B1(ot=0):
                # chunk 0 -> x0..3, 1 -> x4..6(+3), 2 -> x7..9, 3 -> x10..12
                # first chunk split so PE can start after just x0 + W0
                flush_until(xT, 0)
                flush_until(WT, 0)
                b1_span(0, 0, 128)
                flush_until(xT, 3)
                b1_span(0, 128, 266)
                for ci, xts in enumerate(((4, 5, 6), (7, 8, 9), (10, 11, 12))):
                    for tt in xts:
                        load_block(xT, x_ext, tt, 0)
                    flush_until(xT, xts[-1])
                    b1_chunk(0, ci + 1)
                flush_until(WT, 1)
                b1_chunk(1, 0)
                b1_chunk(1, 1)
                for ob in range(2, 24):
                    load_block(WT, qkvw_ext, ob, 0)
                    flush_half(1)
                    if ob < 16:
                        flush_until(WT, ob)
                        b1_otile(ob)
                b1_chunk(1, 2)
                b1_chunk(1, 3)
                flush_half(len(tp_pending))
                while wctr[0] < len(win_jobs):
                    emit_window_set(win_jobs[wctr[0]])
                    wctr[0] += 1

                if stage <= 2:
                    dump(nc, out_ext, xT[0][:, 0:1024], 0)
                    dump(nc, out_ext, WT[0][:, 0:1024], 128)
                    return nc

                # V projection -> V1[(b,kt)] bf16 (+bias)
                for b in range(BC):
                    for kt, (ko, ksz) in enumerate(((0, 128), (128, 69))):
                        t0 = b * NSEQ + ko
                        v1t = V1[(b, kt)]
                        for oc in range(2):
                            pv = psmm.tile([128, 512], F32, name="pv", tag="pq")
                            for ct in range(8):
                                nc.tensor.matmul(
                                    pv[0:ksz, :],
                                    xT[ct][:, t0: t0 + ksz],
                                    WT[ct][:, 2 * C + oc * 512:
                                            2 * C + (oc + 1) * 512],
                                    start=(ct == 0), stop=(ct == 7),
                                )
                            nc.vector.tensor_add(
                                v1t[0:ksz, oc * 512:(oc + 1) * 512],
                                pv[0:ksz, :],
                                vb_bcast[0:ksz, oc * 512:(oc + 1) * 512],
                            )
                        if wrctr[0] < len(win_sets):
                            emit_wrev_extract(wrctr[0])
                            wrctr[0] += 1
                        if wrctr[0] < len(win_sets):
                            emit_wrev_extract(wrctr[0])
                            wrctr[0] += 1
                while wrctr[0] < len(win_sets):
                    emit_wrev_extract(wrctr[0])
                    wrctr[0] += 1

            if stage <= 1 or stage == 4:
                for hh in range(4):
                    dump(nc, out_ext, EBa[hh][:, :], hh * 128)
                return nc

            # ----- phase 2: attention + output projection -------------------
            from contextlib import ExitStack
            with (
                tc.tile_pool(name="attbig", bufs=1) as ab,
                tc.tile_pool(name="stage2", bufs=6) as stage2,
                tc.tile_pool(name="stage2b", bufs=6) as stage2b,
                tc.tile_pool(name="attw", bufs=8) as attw,
                tc.tile_pool(name="attsmall", bufs=6) as attsmall,
                tc.tile_pool(name="ostage", bufs=3) as op_,
            ):
                att_ctx = ExitStack()
                ps_s = att_ctx.enter_context(
                    tc.tile_pool(name="ps_s", bufs=4, space="PSUM"))
                ps_o = att_ctx.enter_context(
                    tc.tile_pool(name="ps_o", bufs=2, space="PSUM"))
                ps_r = att_ctx.enter_context(
                    tc.tile_pool(name="ps_r", bufs=2, space="PSUM"))
                ps_j = ps_r
                attnT = [
                    ab.tile([128, T], BF16, name=f"at{ct}", tag=f"at{ct}")
                    for ct in range(8)
                ]
                PWT = [
                    ab.tile([128, C], BF16, name=f"pwt{ct}", tag=f"pwt{ct}")
                    for ct in range(8)
                ]
                pb_bcast = ab.tile([128, C], F32, name="pb_bcast", tag="pb_bcast")
                for oc in range(2):
                    ppb = ps_r.tile([128, 512], F32, name="pbc2", tag="rbp")
                    nc.tensor.matmul(
                        ppb[:], ones_row[:], pb_row[:, oc * 512:(oc + 1) * 512],
                        start=True, stop=True,
                    )
                    nc.scalar.copy(pb_bcast[:, oc * 512:(oc + 1) * 512], ppb[:])

                pw_jobs = []
                pwdq = [0]
                for ob in range(8):
                    for half in range(2):
                        pwb = stage2.tile([128, 512], F32, name="pwstage",
                                          tag="pwstage")
                        peng = dmaq[pwdq[0] % 3]
                        pwdq[0] += 1
                        peng.dma_start(
                            pwb[:],
                            pw_ext[ob * 128:(ob + 1) * 128,
                                   half * 512:(half + 1) * 512])
                        pwbf = stage2b.tile([128, 512], BF16, name="pwbstage",
                                            tag="pwbstage")
                        nc.vector.tensor_copy(pwbf[:], pwb[:])
                        for ci in range(4):
                            pw_jobs.append((pwbf, ob, half * 4 + ci))
                # stage2 pool must keep pwbf tiles alive until their
                # transposes are emitted (spread through early attention)
                pwctr = [0]

                def emit_pw_transposes(n):
                    while n > 0 and pwctr[0] < len(pw_jobs):
                        (pwbf, ob, ct) = pw_jobs[pwctr[0]]
                        ci = ct % 4
                        pwctr[0] += 1
                        n -= 1
                        ptr = ps_r.tile([128, 128], BF16, name="ptr2", tag="rbp")
                        nc.tensor.transpose(
                            ptr[:], pwbf[:, ci * 128:(ci + 1) * 128], ident_bf[:]
                        )
                        dst = PWT[ct][:, ob * 128:(ob + 1) * 128]
                        if (ob + ct) % 2 == 0:
                            nc.scalar.copy(dst, ptr[:])
                        else:
                            nc.vector.tensor_copy(dst, ptr[:])

                emit_pw_transposes(24)

                b4q = []          # pending out-proj tiles
                b4cur = [None]    # (tt, ost, next_oc)

                def b4_tick():
                    # emit half an out-proj tile (8 matmuls) as PE filler
                    if b4cur[0] is None:
                        if not b4q:
                            return False
                        tt = b4q.pop(0)
                        ost = op_.tile([128, C], F32, name="ost", tag="ost")
                        b4cur[0] = (tt, ost, 0)
                    (tt, ost, oc) = b4cur[0]
                    tsz = min(128, T - tt * 128)
                    pp2 = ps_j.tile([128, 512], F32, name="ppj", tag="rbp")
                    for ct in range(8):
                        nc.tensor.matmul(
                            pp2[0:tsz, :],
                            attnT[ct][:, tt * 128: tt * 128 + tsz],
                            PWT[ct][:, oc * 512:(oc + 1) * 512],
                            start=(ct == 0), stop=(ct == 7),
                        )
                    nc.vector.tensor_add(
                        ost[0:tsz, oc * 512:(oc + 1) * 512],
                        pp2[0:tsz, :],
                        pb_bcast[0:tsz, oc * 512:(oc + 1) * 512],
                    )
                    if oc == 0:
                        b4cur[0] = (tt, ost, 1)
                    else:
                        nc.sync.dma_start(
                            out_ext[tt * 128: tt * 128 + tsz, :], ost[0:tsz, :]
                        )
                        b4cur[0] = None
                    return True

                b4ctr = [0]
                # attention: head-paired column-tiled PV/normalize.
                # All 8 QK matmuls of a block are emitted before the first
                # rowsum so the in-order PE queue never stalls on the
                # exp->mul chain; a half out-proj tile fills the gap.
                for bp in range(BC // 2):
                    t0p = 2 * bp * NSEQ
                    for hp in range(8):
                        po = ps_o.tile([128, 2 * NSEQ], F32, name="po",
                                       tag="po", padded_shape=[128, 512])
                        rbp = ps_r.tile([128, 2 * NSEQ], F32, name="rbp",
                                        tag="rbp", padded_shape=[128, 512])
                        kot = 8 + hp
                        ptd = {}
                        for hh in range(2):
                            h = 2 * hp + hh
                            qpo = 64 * hh
                            for kt, (ko, ksz) in enumerate(((0, 128),
                                                            (128, 69))):
                                ps = ps_s.tile([128, 2 * NSEQ], F32,
                                               name="ps", tag="ps")
                                for bi in range(2):
                                    b = 2 * bp + bi
                                    t0 = b * NSEQ
                                    nc.tensor.matmul(
                                        ps[0:ksz, bi * NSEQ:(bi + 1) * NSEQ],
                                        QKT[kot][qpo: qpo + 64,
                                                 t0 + ko: t0 + ko + ksz],
                                        QKT[hp][qpo: qpo + 64, t0: t0 + NSEQ],
                                        start=True, stop=True,
                                    )
                                pt = attw.tile([128, 2 * NSEQ], BF16,
                                               name="pt", tag="pt")
                                nc.scalar.activation(
                                    pt[0:ksz, :], ps[0:ksz, :], AF.Exp,
                                    scale=SCALE,
                                )
                                eb = (EBa[h] if kt == 0 else EBb[h])
                                for (meng, bi2) in ((nc.vector, 0),
                                                    (nc.gpsimd, 1)):
                                    meng.tensor_mul(
                                        pt[0:ksz, bi2 * NSEQ:(bi2 + 1) * NSEQ],
                                        pt[0:ksz, bi2 * NSEQ:(bi2 + 1) * NSEQ],
                                        eb[0:ksz, :],
                                    )
                                ptd[(hh, kt)] = (pt, ksz)
                        # independent PE work while the exp/mul chains run
                        if not b4_tick():
                            emit_pw_transposes(8)
                        for hh in range(2):
                            h = 2 * hp + hh
                            cpos = 64 * hh
                            for kt in range(2):
                                pt, ksz = ptd[(hh, kt)]
                                nc.tensor.matmul(
                                    rbp[cpos: cpos + 64, :],
                                    onesK[0:ksz, :], pt[0:ksz, :],
                                    start=(kt == 0), stop=(kt == 1),
                                    tile_position=(0, cpos),
                                )
                            for bi in range(2):
                                b = 2 * bp + bi
                                for kt, (ko, ksz) in enumerate(((0, 128),
                                                                (128, 69))):
                                    nc.tensor.matmul(
                                        po[cpos: cpos + 64,
                                           bi * NSEQ:(bi + 1) * NSEQ],
                                        V1[(b, kt)][0:ksz,
                                                    h * HD:(h + 1) * HD],
                                        ptd[(hh, kt)][0][0:ksz,
                                                   bi * NSEQ:(bi + 1) * NSEQ],
                                        start=(kt == 0), stop=(kt == 1),
                                        tile_position=(0, cpos),
                                    )
                        rbs = attsmall.tile([128, 2 * NSEQ], F32,
                                            name="rbs", tag="rbs")
                        nc.vector.reciprocal_approx_fast(rbs[:], rbp[:])
                        nc.vector.tensor_mul(
                            attnT[hp][:, t0p: t0p + 2 * NSEQ],
                            po[:],
                            rbs[:],
                        )
                    while (b4ctr[0] + 1) * 128 <= (bp + 1) * 2 * NSEQ:
                        b4q.append(b4ctr[0])
                        b4ctr[0] += 1

                if stage <= 5:
                    dump(nc, out_ext, attnT[0][:, 0:1024], 0)
                    return nc

                # flush remaining output-projection tiles
                while b4ctr[0] < NT_TILE:
                    b4q.append(b4ctr[0])
                    b4ctr[0] += 1
                while b4q or b4cur[0] is not None:
                    b4_tick()
                att_ctx.close()

    return nc


_NC = None
LAST_RESULT = None


def _get_nc():
    global _NC
    if _NC is None:
        _NC = build_nc()
    return _NC


def make_in_maps(x, qkv_w, q_bias, v_bias, rpb_table, proj_w, proj_b):
    x = np.ascontiguousarray(np.asarray(x, np.float32))
    qkv_w = np.ascontiguousarray(np.asarray(qkv_w, np.float32))
    q_bias = np.ascontiguousarray(np.asarray(q_bias, np.float32).reshape(1, C))
    v_bias = np.ascontiguousarray(np.asarray(v_bias, np.float32).reshape(1, C))
    rpb_table = np.ascontiguousarray(np.asarray(rpb_table, np.float32))
    proj_w = np.ascontiguousarray(np.asarray(proj_w, np.float32))
    proj_b = np.ascontiguousarray(np.asarray(proj_b, np.float32).reshape(1, C))
    in_maps = []
    for c in range(8):
        xs = np.ascontiguousarray(
            x[c * BC:(c + 1) * BC].reshape(T, C)
        )
        in_maps.append({
            "x": xs, "qkv_w": qkv_w, "q_bias": q_bias, "v_bias": v_bias,
            "rpb_table": rpb_table, "proj_w": proj_w, "proj_b": proj_b,
        })
    return in_maps


def _ensure_axon_hooks_importable():
    """bass_utils imports antenv.axon_hooks when BASS_TRACE is set; the image's
    antenv lacks that module. Provide a no-op stand-in so tracing degrades
    gracefully instead of crashing (unless a real one is already installed)."""
    import types
    try:
        import antenv.axon_hooks  # noqa: F401
    except Exception:
        mod = types.ModuleType("antenv.axon_hooks")
        mod._h = None
        mod.set_axon_ntff_profile_hook = lambda h: setattr(mod, "_h", h)
        mod.get_axon_ntff_profile_hook = lambda: mod._h
        sys.modules["antenv.axon_hooks"] = mod
        try:
            import antenv
            antenv.axon_hooks = mod
        except Exception:
            pass


def kernel(x, qkv_w, q_bias, v_bias, rpb_table, proj_w, proj_b,
           rel_pos_index=None, **_unused):
    global LAST_RESULT
    _ensure_axon_hooks_importable()
    from concourse.bass_utils import run_bass_kernel_spmd

    nc = _get_nc()
    in_maps = make_in_maps(x, qkv_w, q_bias, v_bias, rpb_table, proj_w, proj_b)
    res = run_bass_kernel_spmd(nc, in_maps, core_ids=list(range(8)))
    LAST_RESULT = res
    out = np.concatenate(
        [res.results[c]["out"].reshape(BC, NSEQ, C) for c in range(8)], axis=0
    )
    return out.astype(np.float32)
